# revision 4
# baseline (speedup 1.0000x reference)
"""BaiChuan attention layer on 8 Trainium2 NeuronCores.

Sharding: tensor-parallel over heads within groups of 4 cores (W_pack
column-parallel, o_proj column-parallel after a per-head AllGather of
attention outputs), data-parallel over the batch across the two groups.

Per-core dataflow (core c: batch b=c//4, rank r=c%4, heads 8r..8r+8):
  stage A: qkvT[j, t] = W_core @ hs[b].T      (PE, f32r, psum-accumulated)
  stage B: per head: neox RoPE on qT,kT (DVE, swapped-half DMA loads),
           v natural layout via PE transpose, causal attention with
           s^T = kT.T-blocks @ qT (scores transposed), exp on ACT,
           softmax denominator via GPSIMD partition-reduce, PV with
           p^T as moving operand, per-head AllGather of attn outputs
           (overlaps with later heads' compute).
  stage C: o_proj column-parallel: out[t, m_shard] over the full
           (gathered) head dimension.  Host concatenates m-shards.
"""
import sys
sys.path.insert(0, '/opt/trn_rl_repo')
import numpy as np

import concourse.bass as bass
from concourse import bacc
import concourse.mybir as mybir
from concourse.tile import TileContext
from concourse.bass_utils import run_bass_kernel_spmd
from concourse.masks import make_identity

f32 = mybir.dt.float32
f32r = mybir.dt.float32r
AF = mybir.ActivationFunctionType

B, S, H, NH = 2, 2048, 4096, 32
HD = H // NH                    # 128
THETA = 10000.0
NCORES, TPN = 8, 4              # 2 groups of 4 (DP over batch x TP over heads)
HPC = NH // TPN                 # 8 heads per core
JC = HPC * HD                   # 1024 per-core q (=k=v) width
SCALE = HD ** -0.5
GROUPS = [[0, 1, 2, 3], [4, 5, 6, 7]]
TB = 1024                       # stage-A token block
NTB = S // TB
NIB = H // 128                  # 32 contraction blocks
NJT = 3 * JC // 128             # 24 output row-tiles in stage A
NG = S // 512                   # 4 query blocks per head
NKB = S // 128                  # 16 key blocks per head


def build_nc():
    nc = bacc.Bacc(None)
    hsT = nc.declare_dram_parameter("hsT", [H, S], f32, isOutput=False)
    wT = nc.declare_dram_parameter("wT", [H, 3 * JC], f32, isOutput=False)
    woT = nc.declare_dram_parameter("woT", [H, JC], f32, isOutput=False)
    cosf = nc.declare_dram_parameter("cosf", [HD, S], f32, isOutput=False)
    sinm = nc.declare_dram_parameter("sinm", [HD, S], f32, isOutput=False)
    masks = nc.declare_dram_parameter("masks", [4, 128, 512], f32, isOutput=False)
    out = nc.declare_dram_parameter("out", [S, JC], f32, isOutput=True)

    qkvT_d = nc.dram_tensor("qkvT_d", [3 * JC, S], f32)
    attn_d = nc.dram_tensor("attn_d", [HPC, HD, S], f32)
    attn_ag = nc.dram_tensor("attn_ag", [HPC, TPN * HD, S], f32)

    hsT_v = hsT[:].rearrange("(n p) t -> p n t", p=128)      # [128, 32, S]
    wT_v = wT[:].rearrange("(n p) j -> p n j", p=128)        # [128, 32, 3*JC]
    woT_v = woT[:].rearrange("(n p) m -> p n m", p=128)      # [128, 32, JC]
    ag_v = attn_ag[:].rearrange("h (r p) t -> p (h r) t", p=128)  # [128, 32, S]

    with TileContext(nc) as tc:
        # ---------------- stage A: fused QKV projection ----------------
        with nc.named_scope("stageA"), \
             tc.tile_pool(name="stA", bufs=1) as pa, \
             tc.tile_pool(name="psA", bufs=4, space="PSUM") as psA:
            for tb in range(NTB):
                hs_sb = pa.tile([128, NIB, TB], f32r, tag="hs", bufs=1,
                                name=f"hs_{tb}")
                for d in range(8):
                    nc.sync.dma_start(
                        out=hs_sb[:, 4 * d:4 * (d + 1), :],
                        in_=hsT_v[:, 4 * d:4 * (d + 1),
                                  tb * TB:(tb + 1) * TB].bitcast(f32r))
                for jt in range(NJT):
                    w_sb = pa.tile([128, NIB, 128], f32r, tag="w", bufs=3,
                                   name=f"w_{tb}_{jt}")
                    nc.sync.dma_start(
                        out=w_sb[:],
                        in_=wT_v[:, :, jt * 128:(jt + 1) * 128].bitcast(f32r))
                    for th in range(TB // 512):
                        ps = psA.tile([128, 512], f32, tag="psA",
                                      name=f"psA_{tb}_{jt}_{th}")
                        for ib in range(NIB):
                            nc.tensor.matmul(
                                ps[:], w_sb[:, ib, :],
                                hs_sb[:, ib, th * 512:(th + 1) * 512],
                                start=(ib == 0), stop=(ib == NIB - 1))
                        st = pa.tile([128, 512], f32, tag="oA", bufs=4,
                                     name=f"stA_{tb}_{jt}_{th}")
                        nc.scalar.copy(st[:], ps[:])
                        nc.sync.dma_start(
                            out=qkvT_d[:][jt * 128:(jt + 1) * 128,
                                          tb * TB + th * 512:
                                          tb * TB + (th + 1) * 512],
                            in_=st[:])

        # ---------------- stage B: rope + causal attention ----------------
        with nc.named_scope("stageB"), \
             tc.tile_pool(name="stB", bufs=1) as pb, \
             tc.tile_pool(name="psB", bufs=1, space="PSUM") as psB:
            ident = pb.tile([128, 128], f32, tag="ident", bufs=1)
            make_identity(nc, ident[:])
            ones_f = pb.tile([1, 128], f32, tag="ones_f", bufs=1)
            nc.vector.memset(ones_f[:], 1.0)
            ones_r = pb.tile([1, 128], f32r, tag="ones_r", bufs=1)
            nc.vector.tensor_copy(ones_r[:], ones_f[:])
            cos_sb = pb.tile([128, S], f32, tag="cos", bufs=1)
            sin_sb = pb.tile([128, S], f32, tag="sin", bufs=1)
            nc.sync.dma_start(out=cos_sb[:], in_=cosf[:])
            nc.sync.dma_start(out=sin_sb[:], in_=sinm[:])
            mask_sb = pb.tile([128, 4, 512], f32, tag="mask", bufs=1)
            nc.sync.dma_start(out=mask_sb[:],
                              in_=masks[:].rearrange("v p x -> p v x"))

            def load_rope(jt, name):
                """load qkvT_d row-block jt, apply neox rope, emit f32r tile"""
                raw = pb.tile([128, S], f32, tag="raw", bufs=4,
                              name=f"{name}_raw")
                nc.sync.dma_start(out=raw[:],
                                  in_=qkvT_d[:][jt * 128:(jt + 1) * 128, :])
                sw = pb.tile([128, S], f32, tag="raw", bufs=4,
                             name=f"{name}_sw")
                nc.sync.dma_start(out=sw[0:64, :],
                                  in_=qkvT_d[:][jt * 128 + 64:jt * 128 + 128, :])
                nc.sync.dma_start(out=sw[64:128, :],
                                  in_=qkvT_d[:][jt * 128:jt * 128 + 64, :])
                t1 = pb.tile([128, S], f32, tag="ropetmp", bufs=2,
                             name=f"{name}_t1")
                t2 = pb.tile([128, S], f32, tag="ropetmp", bufs=2,
                             name=f"{name}_t2")
                nc.vector.tensor_mul(t1[:], raw[:], cos_sb[:])
                nc.vector.tensor_mul(t2[:], sw[:], sin_sb[:])
                rt = pb.tile([128, S], f32r, tag=f"{name}_r", bufs=2,
                             name=f"{name}_roped")
                nc.vector.tensor_add(rt[:], t1[:], t2[:])
                return rt

            for h in range(HPC):
                with nc.named_scope(f"head{h}"):
                    kT = load_rope(HPC + h, "k")
                    qT = load_rope(h, "q")
                    vraw = pb.tile([128, S], f32, tag="raw", bufs=4,
                                   name=f"vraw_{h}")
                    nc.sync.dma_start(
                        out=vraw[:],
                        in_=qkvT_d[:][(2 * HPC + h) * 128:
                                      (2 * HPC + h + 1) * 128, :])
                    v_sb = pb.tile([128, NKB, 128], f32r, tag="vsb", bufs=2,
                                   name=f"v_{h}")
                    for kb in range(NKB):
                        pst = psB.tile([128, 512], f32, tag="pss", bufs=3,
                                       name=f"ptr_{h}_{kb}")
                        nc.tensor.transpose(pst[0:128, 0:128],
                                            vraw[:, kb * 128:(kb + 1) * 128],
                                            ident[:])
                        nc.scalar.copy(v_sb[:, kb, :], pst[0:128, 0:128])

                    attn = pb.tile([128, S], f32r, tag="attn", bufs=2,
                                   name=f"attn_{h}")
                    for g in range(NG):
                        nkb = 4 * g + 4
                        po = psB.tile([128, 512], f32, tag="po", bufs=2,
                                      name=f"po_{h}_{g}")
                        rs_acc = pb.tile([1, 512], f32, tag="rsacc", bufs=2,
                                         name=f"rsacc_{h}_{g}")
                        for kb in range(nkb):
                            pss = psB.tile([128, 512], f32, tag="pss", bufs=3,
                                           name=f"pss_{h}_{g}_{kb}")
                            nc.tensor.matmul(
                                pss[:], kT[:, kb * 128:(kb + 1) * 128],
                                qT[:, g * 512:(g + 1) * 512],
                                start=True, stop=True)
                            pt = pb.tile([128, 512], f32r, tag="pt", bufs=3,
                                         name=f"pt_{h}_{g}_{kb}")
                            nc.scalar.activation(pt[:], pss[:], AF.Exp,
                                                 scale=SCALE)
                            if kb >= 4 * g:
                                nc.vector.tensor_mul(pt[:], pt[:],
                                                     mask_sb[:, kb - 4 * g, :])
                            rs_p = pb.tile([1, 512], f32, tag="rsp", bufs=3,
                                           name=f"rsp_{h}_{g}_{kb}")
                            nc.gpsimd.tensor_reduce(
                                rs_p[:], pt[:].bitcast(f32),
                                mybir.AxisListType.C, mybir.AluOpType.add)
                            if kb == 0:
                                nc.vector.tensor_copy(rs_acc[:], rs_p[:])
                            else:
                                nc.vector.tensor_add(rs_acc[:], rs_acc[:],
                                                     rs_p[:])
                            nc.tensor.matmul(po[:], v_sb[:, kb, :], pt[:],
                                             start=(kb == 0),
                                             stop=(kb == nkb - 1))
                        rr = pb.tile([1, 512], f32, tag="rr", bufs=2,
                                     name=f"rr_{h}_{g}")
                        nc.vector.reciprocal(rr[:], rs_acc[:])
                        rr_r = pb.tile([1, 512], f32r, tag="rrr", bufs=2,
                                       name=f"rrr_{h}_{g}")
                        nc.vector.tensor_copy(rr_r[:], rr[:])
                        pbc = psB.tile([128, 512], f32, tag="pb", bufs=2,
                                       name=f"pbc_{h}_{g}")
                        nc.tensor.matmul(pbc[:], ones_r[:], rr_r[:],
                                         start=True, stop=True)
                        bc = pb.tile([128, 512], f32, tag="bc", bufs=2,
                                     name=f"bc_{h}_{g}")
                        nc.scalar.copy(bc[:], pbc[:])
                        nc.vector.tensor_mul(attn[:, g * 512:(g + 1) * 512],
                                             po[:], bc[:])
                    nc.sync.dma_start(out=attn_d[:][h], in_=attn[:].bitcast(f32))
                    nc.gpsimd.collective_compute(
                        "AllGather", mybir.AluOpType.bypass,
                        replica_groups=GROUPS,
                        ins=[attn_d[:][h]], outs=[attn_ag[:][h]])

        # ---------------- stage C: o_proj (column-parallel) ----------------
        with nc.named_scope("stageC"), \
             tc.tile_pool(name="stC", bufs=1) as pc, \
             tc.tile_pool(name="psC", bufs=4, space="PSUM") as psC:
            wo_sb = pc.tile([128, NIB, JC], f32r, tag="wo", bufs=1)
            for d in range(8):
                nc.sync.dma_start(
                    out=wo_sb[:, 4 * d:4 * (d + 1), :],
                    in_=woT_v[:, 4 * d:4 * (d + 1), :].bitcast(f32r))
            for tb in range(NKB):
                at_sb = pc.tile([128, NIB, 128], f32r, tag="atC", bufs=2,
                                name=f"atC_{tb}")
                nc.sync.dma_start(
                    out=at_sb[:],
                    in_=ag_v[:, :, tb * 128:(tb + 1) * 128].bitcast(f32r))
                for mc in range(JC // 512):
                    psc = psC.tile([128, 512], f32, tag="psC",
                                   name=f"psC_{tb}_{mc}")
                    for jb in range(NIB):
                        nc.tensor.matmul(
                            psc[:], at_sb[:, jb, :],
                            wo_sb[:, jb, mc * 512:(mc + 1) * 512],
                            start=(jb == 0), stop=(jb == NIB - 1))
                    oc = pc.tile([128, 512], f32, tag="oC", bufs=4,
                                 name=f"oC_{tb}_{mc}")
                    nc.scalar.copy(oc[:], psc[:])
                    nc.sync.dma_start(
                        out=out[:][tb * 128:(tb + 1) * 128,
                                   mc * 512:(mc + 1) * 512],
                        in_=oc[:])

    nc.finalize()
    return nc


_NC_CACHE = None


def _get_nc():
    global _NC_CACHE
    if _NC_CACHE is None:
        _NC_CACHE = build_nc()
    return _NC_CACHE


def _host_inputs(hidden_states, positions, w_pack, w_o):
    hidden_states = np.asarray(hidden_states, dtype=np.float32)
    positions = np.asarray(positions)
    w_pack = np.asarray(w_pack, dtype=np.float32)
    w_o = np.asarray(w_o, dtype=np.float32)

    half = HD // 2
    inv_freq = (1.0 / (THETA ** (np.arange(half, dtype=np.float32) / half)))

    # causal mask variants for the 4 diagonal (128x512) tiles of a q-block
    masks = np.empty((4, 128, 512), dtype=np.float32)
    xs = np.arange(512)[None, :]
    ps = np.arange(128)[:, None]
    for v in range(4):
        masks[v] = (xs >= ps + 128 * v).astype(np.float32)

    in_maps = []
    for c in range(NCORES):
        b, r = divmod(c, TPN)
        heads = np.arange(HPC * r, HPC * (r + 1))
        rows = (heads[:, None] * HD + np.arange(HD)[None, :]).reshape(-1)
        w_core = np.concatenate(
            [w_pack[rows], w_pack[H + rows], w_pack[2 * H + rows]], axis=0)
        wT = np.ascontiguousarray(w_core.T)                      # [H, 3*JC]
        # o_proj m-shard rows, j-order permuted to match AllGather layout:
        # gathered row (h, r', d) holds global head 8*r'+h
        wo_shard = w_o[JC * r:JC * (r + 1), :]                   # [JC, H]
        woT_full = np.ascontiguousarray(wo_shard.T)              # [H=j, JC]
        woT_perm = woT_full.reshape(TPN, HPC, HD, JC) \
                           .transpose(1, 0, 2, 3).reshape(H, JC)
        hsT = np.ascontiguousarray(hidden_states[b].T)           # [H, S]
        ang = positions[b].astype(np.float32)[None, :] * inv_freq[:, None]
        cos_t = np.cos(ang).astype(np.float32)                   # [64, S]
        sin_t = np.sin(ang).astype(np.float32)
        cosf = np.concatenate([cos_t, cos_t], axis=0)            # [128, S]
        sinm = np.concatenate([-sin_t, sin_t], axis=0)
        in_maps.append({
            "hsT": hsT, "wT": wT, "woT": np.ascontiguousarray(woT_perm),
            "cosf": cosf, "sinm": sinm, "masks": masks,
        })
    return in_maps


def kernel(hidden_states, positions, w_pack, w_o):
    nc = _get_nc()
    in_maps = _host_inputs(hidden_states, positions, w_pack, w_o)
    res = run_bass_kernel_spmd(nc, in_maps, list(range(NCORES)))
    out = np.empty((B, S, H), dtype=np.float32)
    for c in range(NCORES):
        b, r = divmod(c, TPN)
        out[b][:, JC * r:JC * (r + 1)] = res.results[c]["out"]
    return out


# revision 6
# speedup vs baseline: 11.3766x; 11.3766x over previous
"""BaiChuan attention layer on 8 Trainium2 NeuronCores.

Sharding: tensor-parallel over heads within groups of 4 cores (W_pack
column-parallel, o_proj column-parallel after a per-head AllGather of
attention outputs), data-parallel over the batch across the two groups.

Per-core dataflow (core c: batch b=c//4, rank r=c%4, heads 8r..8r+8):
  stage A: qkvT[j, t] = W_core @ hs[b].T      (PE, f32r, psum-accumulated)
  stage B: per head: neox RoPE on qT,kT (DVE, swapped-half DMA loads),
           v natural layout via PE transpose, causal attention with
           s^T = kT.T-blocks @ qT (scores transposed), exp on ACT,
           softmax denominator via GPSIMD partition-reduce, PV with
           p^T as moving operand, per-head AllGather of attn outputs
           (overlaps with later heads' compute).
  stage C: o_proj column-parallel: out[t, m_shard] over the full
           (gathered) head dimension.  Host concatenates m-shards.
"""
import sys
sys.path.insert(0, '/opt/trn_rl_repo')
import numpy as np

import concourse.bass as bass
from concourse import bacc
import concourse.mybir as mybir
from concourse.tile import TileContext
from concourse.bass_utils import run_bass_kernel_spmd
from concourse.masks import make_identity
from concourse import bass_isa

f32 = mybir.dt.float32
f32r = mybir.dt.float32r
AF = mybir.ActivationFunctionType

B, S, H, NH = 2, 2048, 4096, 32
HD = H // NH                    # 128
THETA = 10000.0
NCORES, TPN = 8, 4              # 2 groups of 4 (DP over batch x TP over heads)
HPC = NH // TPN                 # 8 heads per core
JC = HPC * HD                   # 1024 per-core q (=k=v) width
SCALE = HD ** -0.5
GROUPS = [[0, 1, 2, 3], [4, 5, 6, 7]]
TB = 1024                       # stage-A token block
NTB = S // TB
NIB = H // 128                  # 32 contraction blocks
NJT = 3 * JC // 128             # 24 output row-tiles in stage A
NG = S // 512                   # 4 query blocks per head
NKB = S // 128                  # 16 key blocks per head


def build_nc():
    nc = bacc.Bacc(None)
    hsT = nc.declare_dram_parameter("hsT", [H, S], f32, isOutput=False)
    wT = nc.declare_dram_parameter("wT", [H, 3 * JC], f32, isOutput=False)
    woT = nc.declare_dram_parameter("woT", [H, JC], f32, isOutput=False)
    cosf = nc.declare_dram_parameter("cosf", [HD, S], f32, isOutput=False)
    sinm = nc.declare_dram_parameter("sinm", [HD, S], f32, isOutput=False)
    masks = nc.declare_dram_parameter("masks", [4, 128, 512], f32, isOutput=False)
    out = nc.declare_dram_parameter("out", [S, JC], f32, isOutput=True)

    qkvT_d = nc.dram_tensor("qkvT_d", [3 * JC, S], f32)
    attn_d = nc.dram_tensor("attn_d", [HPC, HD, S], f32)
    attn_ag = nc.dram_tensor("attn_ag", [HPC, TPN * HD, S], f32)

    hsT_v = hsT[:].rearrange("(n p) t -> p n t", p=128)      # [128, 32, S]
    wT_v = wT[:].rearrange("(n p) j -> p n j", p=128)        # [128, 32, 3*JC]
    woT_v = woT[:].rearrange("(n p) m -> p n m", p=128)      # [128, 32, JC]
    ag_v = attn_ag[:].rearrange("h (r p) t -> p (h r) t", p=128)  # [128, 32, S]

    with TileContext(nc) as tc:
        # ---------------- stage A: fused QKV projection ----------------
        with nc.named_scope("stageA"), \
             tc.tile_pool(name="stA", bufs=1) as pa, \
             tc.tile_pool(name="psA", bufs=4, space="PSUM") as psA:
            for tb in range(NTB):
                hs_sb = pa.tile([128, NIB, TB], f32r, tag="hs", bufs=1,
                                name=f"hs_{tb}")
                for d in range(8):
                    nc.sync.dma_start(
                        out=hs_sb[:, 4 * d:4 * (d + 1), :],
                        in_=hsT_v[:, 4 * d:4 * (d + 1),
                                  tb * TB:(tb + 1) * TB].bitcast(f32r))
                for jt in range(NJT):
                    w_sb = pa.tile([128, NIB, 128], f32r, tag="w", bufs=3,
                                   name=f"w_{tb}_{jt}")
                    nc.sync.dma_start(
                        out=w_sb[:],
                        in_=wT_v[:, :, jt * 128:(jt + 1) * 128].bitcast(f32r))
                    for th in range(TB // 512):
                        ps = psA.tile([128, 512], f32, tag="psA",
                                      name=f"psA_{tb}_{jt}_{th}")
                        for ib in range(NIB):
                            nc.tensor.matmul(
                                ps[:], w_sb[:, ib, :],
                                hs_sb[:, ib, th * 512:(th + 1) * 512],
                                start=(ib == 0), stop=(ib == NIB - 1))
                        st = pa.tile([128, 512], f32, tag="oA", bufs=4,
                                     name=f"stA_{tb}_{jt}_{th}")
                        nc.scalar.copy(st[:], ps[:])
                        nc.sync.dma_start(
                            out=qkvT_d[:][jt * 128:(jt + 1) * 128,
                                          tb * TB + th * 512:
                                          tb * TB + (th + 1) * 512],
                            in_=st[:])

        # ---------------- stage B: rope + causal attention ----------------
        with nc.named_scope("stageB"), \
             tc.tile_pool(name="stB", bufs=1) as pb, \
             tc.tile_pool(name="psB", bufs=1, space="PSUM") as psB:
            ident = pb.tile([128, 128], f32, tag="ident", bufs=1)
            make_identity(nc, ident[:])
            cos_sb = pb.tile([128, S], f32, tag="cos", bufs=1)
            sin_sb = pb.tile([128, S], f32, tag="sin", bufs=1)
            nc.sync.dma_start(out=cos_sb[:], in_=cosf[:])
            nc.sync.dma_start(out=sin_sb[:], in_=sinm[:])
            mask_sb = pb.tile([128, 4, 512], f32, tag="mask", bufs=1)
            nc.sync.dma_start(out=mask_sb[:],
                              in_=masks[:].rearrange("v p x -> p v x"))

            def load_rope(jt, name):
                """load qkvT_d row-block jt, apply neox rope, emit f32r tile"""
                raw = pb.tile([128, S], f32, tag="raw", bufs=4,
                              name=f"{name}_raw")
                nc.sync.dma_start(out=raw[:],
                                  in_=qkvT_d[:][jt * 128:(jt + 1) * 128, :])
                sw = pb.tile([128, S], f32, tag="raw", bufs=4,
                             name=f"{name}_sw")
                nc.sync.dma_start(out=sw[0:64, :],
                                  in_=qkvT_d[:][jt * 128 + 64:jt * 128 + 128, :])
                nc.sync.dma_start(out=sw[64:128, :],
                                  in_=qkvT_d[:][jt * 128:jt * 128 + 64, :])
                t1 = pb.tile([128, S], f32, tag="ropetmp", bufs=2,
                             name=f"{name}_t1")
                t2 = pb.tile([128, S], f32, tag="ropetmp", bufs=2,
                             name=f"{name}_t2")
                nc.vector.tensor_mul(t1[:], raw[:], cos_sb[:])
                nc.vector.tensor_mul(t2[:], sw[:], sin_sb[:])
                rt = pb.tile([128, S], f32r, tag=f"{name}_r", bufs=2,
                             name=f"{name}_roped")
                nc.vector.tensor_add(rt[:], t1[:], t2[:])
                return rt

            for h in range(HPC):
                with nc.named_scope(f"head{h}"):
                    kT = load_rope(HPC + h, "k")
                    qT = load_rope(h, "q")
                    vraw = pb.tile([128, S], f32, tag="raw", bufs=4,
                                   name=f"vraw_{h}")
                    nc.sync.dma_start(
                        out=vraw[:],
                        in_=qkvT_d[:][(2 * HPC + h) * 128:
                                      (2 * HPC + h + 1) * 128, :])
                    v_sb = pb.tile([128, NKB, 128], f32r, tag="vsb", bufs=2,
                                   name=f"v_{h}")
                    for kb in range(NKB):
                        pst = psB.tile([128, 512], f32, tag="pss", bufs=3,
                                       name=f"ptr_{h}_{kb}")
                        nc.tensor.transpose(pst[0:128, 0:128],
                                            vraw[:, kb * 128:(kb + 1) * 128],
                                            ident[:])
                        nc.scalar.copy(v_sb[:, kb, :], pst[0:128, 0:128])

                    attn = pb.tile([128, S], f32r, tag="attn", bufs=2,
                                   name=f"attn_{h}")
                    for g in range(NG):
                        nkb = 4 * g + 4
                        po = psB.tile([128, 512], f32, tag="po", bufs=2,
                                      name=f"po_{h}_{g}")
                        sacc = pb.tile([128, 512], f32, tag="sacc", bufs=2,
                                       name=f"sacc_{h}_{g}")
                        for kb in range(nkb):
                            pss = psB.tile([128, 512], f32, tag="pss", bufs=3,
                                           name=f"pss_{h}_{g}_{kb}")
                            nc.tensor.matmul(
                                pss[:], kT[:, kb * 128:(kb + 1) * 128],
                                qT[:, g * 512:(g + 1) * 512],
                                start=True, stop=True)
                            pt = pb.tile([128, 512], f32r, tag="pt", bufs=3,
                                         name=f"pt_{h}_{g}_{kb}")
                            nc.scalar.activation(pt[:], pss[:], AF.Exp,
                                                 scale=SCALE)
                            if kb >= 4 * g:
                                nc.vector.tensor_mul(pt[:], pt[:],
                                                     mask_sb[:, kb - 4 * g, :])
                            if kb == 0:
                                nc.vector.tensor_copy(sacc[:],
                                                      pt[:].bitcast(f32))
                            else:
                                nc.vector.tensor_add(sacc[:], sacc[:],
                                                     pt[:].bitcast(f32))
                            nc.tensor.matmul(po[:], v_sb[:, kb, :], pt[:],
                                             start=(kb == 0),
                                             stop=(kb == nkb - 1))
                        den = pb.tile([128, 512], f32, tag="den", bufs=2,
                                      name=f"den_{h}_{g}")
                        nc.gpsimd.partition_all_reduce(
                            den[:], sacc[:], 128, bass_isa.ReduceOp.add)
                        rden = pb.tile([128, 512], f32, tag="rden", bufs=2,
                                       name=f"rden_{h}_{g}")
                        nc.vector.reciprocal(rden[:], den[:])
                        nc.vector.tensor_mul(attn[:, g * 512:(g + 1) * 512],
                                             po[:], rden[:])
                    nc.sync.dma_start(out=attn_d[:][h], in_=attn[:].bitcast(f32))
                    nc.gpsimd.collective_compute(
                        "AllGather", mybir.AluOpType.bypass,
                        replica_groups=GROUPS,
                        ins=[attn_d[:][h]], outs=[attn_ag[:][h]])

        # ---------------- stage C: o_proj (column-parallel) ----------------
        with nc.named_scope("stageC"), \
             tc.tile_pool(name="stC", bufs=1) as pc, \
             tc.tile_pool(name="psC", bufs=4, space="PSUM") as psC:
            wo_sb = pc.tile([128, NIB, JC], f32r, tag="wo", bufs=1)
            for d in range(8):
                nc.sync.dma_start(
                    out=wo_sb[:, 4 * d:4 * (d + 1), :],
                    in_=woT_v[:, 4 * d:4 * (d + 1), :].bitcast(f32r))
            for tb in range(NKB):
                at_sb = pc.tile([128, NIB, 128], f32r, tag="atC", bufs=2,
                                name=f"atC_{tb}")
                nc.sync.dma_start(
                    out=at_sb[:],
                    in_=ag_v[:, :, tb * 128:(tb + 1) * 128].bitcast(f32r))
                for mc in range(JC // 512):
                    psc = psC.tile([128, 512], f32, tag="psC",
                                   name=f"psC_{tb}_{mc}")
                    for jb in range(NIB):
                        nc.tensor.matmul(
                            psc[:], at_sb[:, jb, :],
                            wo_sb[:, jb, mc * 512:(mc + 1) * 512],
                            start=(jb == 0), stop=(jb == NIB - 1))
                    oc = pc.tile([128, 512], f32, tag="oC", bufs=4,
                                 name=f"oC_{tb}_{mc}")
                    nc.scalar.copy(oc[:], psc[:])
                    nc.sync.dma_start(
                        out=out[:][tb * 128:(tb + 1) * 128,
                                   mc * 512:(mc + 1) * 512],
                        in_=oc[:])

    nc.finalize()
    return nc


_NC_CACHE = None


def _get_nc():
    global _NC_CACHE
    if _NC_CACHE is None:
        _NC_CACHE = build_nc()
    return _NC_CACHE


def _host_inputs(hidden_states, positions, w_pack, w_o):
    hidden_states = np.asarray(hidden_states, dtype=np.float32)
    positions = np.asarray(positions)
    w_pack = np.asarray(w_pack, dtype=np.float32)
    w_o = np.asarray(w_o, dtype=np.float32)

    half = HD // 2
    inv_freq = (1.0 / (THETA ** (np.arange(half, dtype=np.float32) / half)))

    # causal mask variants for the 4 diagonal (128x512) tiles of a q-block
    masks = np.empty((4, 128, 512), dtype=np.float32)
    xs = np.arange(512)[None, :]
    ps = np.arange(128)[:, None]
    for v in range(4):
        masks[v] = (xs >= ps + 128 * v).astype(np.float32)

    in_maps = []
    for c in range(NCORES):
        b, r = divmod(c, TPN)
        heads = np.arange(HPC * r, HPC * (r + 1))
        rows = (heads[:, None] * HD + np.arange(HD)[None, :]).reshape(-1)
        w_core = np.concatenate(
            [w_pack[rows], w_pack[H + rows], w_pack[2 * H + rows]], axis=0)
        wT = np.ascontiguousarray(w_core.T)                      # [H, 3*JC]
        # o_proj m-shard rows, j-order permuted to match AllGather layout:
        # gathered row (h, r', d) holds global head 8*r'+h
        wo_shard = w_o[JC * r:JC * (r + 1), :]                   # [JC, H]
        woT_full = np.ascontiguousarray(wo_shard.T)              # [H=j, JC]
        woT_perm = woT_full.reshape(TPN, HPC, HD, JC) \
                           .transpose(1, 0, 2, 3).reshape(H, JC)
        hsT = np.ascontiguousarray(hidden_states[b].T)           # [H, S]
        ang = positions[b].astype(np.float32)[None, :] * inv_freq[:, None]
        cos_t = np.cos(ang).astype(np.float32)                   # [64, S]
        sin_t = np.sin(ang).astype(np.float32)
        cosf = np.concatenate([cos_t, cos_t], axis=0)            # [128, S]
        sinm = np.concatenate([-sin_t, sin_t], axis=0)
        in_maps.append({
            "hsT": hsT, "wT": wT, "woT": np.ascontiguousarray(woT_perm),
            "cosf": cosf, "sinm": sinm, "masks": masks,
        })
    return in_maps


def kernel(hidden_states, positions, w_pack, w_o):
    nc = _get_nc()
    in_maps = _host_inputs(hidden_states, positions, w_pack, w_o)
    res = run_bass_kernel_spmd(nc, in_maps, list(range(NCORES)))
    out = np.empty((B, S, H), dtype=np.float32)
    for c in range(NCORES):
        b, r = divmod(c, TPN)
        out[b][:, JC * r:JC * (r + 1)] = res.results[c]["out"]
    return out


# revision 8
# speedup vs baseline: 11.9682x; 1.0520x over previous
"""BaiChuan attention layer on 8 Trainium2 NeuronCores.

Sharding: tensor-parallel over heads within groups of 4 cores (W_pack
column-parallel, o_proj column-parallel after a per-head AllGather of
attention outputs), data-parallel over the batch across the two groups.

Per-core dataflow (core c: batch b=c//4, rank r=c%4, heads 8r..8r+8):
  stage A: qkvT[j, t] = W_core @ hs[b].T      (PE, f32r, psum-accumulated)
  stage B: per head: neox RoPE on qT,kT (DVE, swapped-half DMA loads),
           v natural layout via PE transpose, causal attention with
           s^T = kT.T-blocks @ qT (scores transposed), exp on ACT,
           softmax denominator via GPSIMD partition-reduce, PV with
           p^T as moving operand, per-head AllGather of attn outputs
           (overlaps with later heads' compute).
  stage C: o_proj column-parallel: out[t, m_shard] over the full
           (gathered) head dimension.  Host concatenates m-shards.
"""
import sys
sys.path.insert(0, '/opt/trn_rl_repo')
import numpy as np

import concourse.bass as bass
from concourse import bacc
import concourse.mybir as mybir
from concourse.tile import TileContext
from concourse.bass_utils import run_bass_kernel_spmd
from concourse.masks import make_identity
from concourse import bass_isa

f32 = mybir.dt.float32
f32r = mybir.dt.float32r
AF = mybir.ActivationFunctionType

B, S, H, NH = 2, 2048, 4096, 32
HD = H // NH                    # 128
THETA = 10000.0
NCORES, TPN = 8, 4              # 2 groups of 4 (DP over batch x TP over heads)
HPC = NH // TPN                 # 8 heads per core
JC = HPC * HD                   # 1024 per-core q (=k=v) width
SCALE = HD ** -0.5
GROUPS = [[0, 1, 2, 3], [4, 5, 6, 7]]
TB = 1024                       # stage-A token block
NTB = S // TB
NIB = H // 128                  # 32 contraction blocks
NJT = 3 * JC // 128             # 24 output row-tiles in stage A
NG = S // 512                   # 4 query blocks per head
NKB = S // 128                  # 16 key blocks per head


def build_nc():
    nc = bacc.Bacc(None)
    hsT = nc.declare_dram_parameter("hsT", [H, S], f32, isOutput=False)
    wT = nc.declare_dram_parameter("wT", [H, 3 * JC], f32, isOutput=False)
    woT = nc.declare_dram_parameter("woT", [H, JC], f32, isOutput=False)
    cosf = nc.declare_dram_parameter("cosf", [HD, S], f32, isOutput=False)
    sinm = nc.declare_dram_parameter("sinm", [HD, S], f32, isOutput=False)
    masks = nc.declare_dram_parameter("masks", [4, 128, 512], f32, isOutput=False)
    out = nc.declare_dram_parameter("out", [S, JC], f32, isOutput=True)

    qkvT_d = nc.dram_tensor("qkvT_d", [3 * JC, S], f32)
    attn_d = nc.dram_tensor("attn_d", [HPC, HD, S], f32)
    attn_ag = nc.dram_tensor("attn_ag", [HPC, TPN * HD, S], f32)

    hsT_v = hsT[:].rearrange("(n p) t -> p n t", p=128)      # [128, 32, S]
    wT_v = wT[:].rearrange("(n p) j -> p n j", p=128)        # [128, 32, 3*JC]
    woT_v = woT[:].rearrange("(n p) m -> p n m", p=128)      # [128, 32, JC]
    ag_v = attn_ag[:].rearrange("h (r p) t -> p (h r) t", p=128)  # [128, 32, S]

    with TileContext(nc) as tc:
        # ---------------- stage A: fused QKV projection ----------------
        with nc.named_scope("stageA"), \
             tc.tile_pool(name="stA", bufs=1) as pa, \
             tc.tile_pool(name="psA", bufs=4, space="PSUM") as psA:
            for tb in range(NTB):
                hs_sb = pa.tile([128, NIB, TB], f32r, tag="hs", bufs=1,
                                name=f"hs_{tb}")
                for d in range(8):
                    nc.sync.dma_start(
                        out=hs_sb[:, 4 * d:4 * (d + 1), :],
                        in_=hsT_v[:, 4 * d:4 * (d + 1),
                                  tb * TB:(tb + 1) * TB].bitcast(f32r))
                for jt in range(NJT):
                    w_sb = pa.tile([128, NIB, 128], f32r, tag="w", bufs=3,
                                   name=f"w_{tb}_{jt}")
                    nc.sync.dma_start(
                        out=w_sb[:],
                        in_=wT_v[:, :, jt * 128:(jt + 1) * 128].bitcast(f32r))
                    for th in range(TB // 512):
                        ps = psA.tile([128, 512], f32, tag="psA",
                                      name=f"psA_{tb}_{jt}_{th}")
                        for ib in range(NIB):
                            nc.tensor.matmul(
                                ps[:], w_sb[:, ib, :],
                                hs_sb[:, ib, th * 512:(th + 1) * 512],
                                start=(ib == 0), stop=(ib == NIB - 1))
                        st = pa.tile([128, 512], f32, tag="oA", bufs=4,
                                     name=f"stA_{tb}_{jt}_{th}")
                        nc.scalar.copy(st[:], ps[:])
                        nc.sync.dma_start(
                            out=qkvT_d[:][jt * 128:(jt + 1) * 128,
                                          tb * TB + th * 512:
                                          tb * TB + (th + 1) * 512],
                            in_=st[:])

        # ---------------- stage B: rope + causal attention ----------------
        with nc.named_scope("stageB"), \
             tc.tile_pool(name="stB", bufs=1) as pb, \
             tc.tile_pool(name="psB", bufs=1, space="PSUM") as psB:
            ident = pb.tile([128, 128], f32, tag="ident", bufs=1)
            make_identity(nc, ident[:])
            ones_f = pb.tile([128, 1], f32, tag="ones_f", bufs=1)
            nc.vector.memset(ones_f[:], 1.0)
            ones_r = pb.tile([128, 1], f32r, tag="ones_r", bufs=1)
            nc.vector.tensor_copy(ones_r[:], ones_f[:])
            cos_sb = pb.tile([128, S], f32, tag="cos", bufs=1)
            sin_sb = pb.tile([128, S], f32, tag="sin", bufs=1)
            nc.sync.dma_start(out=cos_sb[:], in_=cosf[:])
            nc.sync.dma_start(out=sin_sb[:], in_=sinm[:])
            mask_sb = pb.tile([128, 4, 512], f32, tag="mask", bufs=1)
            nc.sync.dma_start(out=mask_sb[:],
                              in_=masks[:].rearrange("v p x -> p v x"))

            def load_rope(jt, name):
                """load qkvT_d row-block jt, apply neox rope, emit f32r tile"""
                raw = pb.tile([128, S], f32, tag="raw", bufs=4,
                              name=f"{name}_raw")
                nc.sync.dma_start(out=raw[:],
                                  in_=qkvT_d[:][jt * 128:(jt + 1) * 128, :])
                sw = pb.tile([128, S], f32, tag="raw", bufs=4,
                             name=f"{name}_sw")
                nc.sync.dma_start(out=sw[0:64, :],
                                  in_=qkvT_d[:][jt * 128 + 64:jt * 128 + 128, :])
                nc.sync.dma_start(out=sw[64:128, :],
                                  in_=qkvT_d[:][jt * 128:jt * 128 + 64, :])
                t1 = pb.tile([128, S], f32, tag="ropetmp", bufs=2,
                             name=f"{name}_t1")
                t2 = pb.tile([128, S], f32, tag="ropetmp", bufs=2,
                             name=f"{name}_t2")
                nc.vector.tensor_mul(t1[:], raw[:], cos_sb[:])
                nc.vector.tensor_mul(t2[:], sw[:], sin_sb[:])
                rt = pb.tile([128, S], f32r, tag=f"{name}_r", bufs=2,
                             name=f"{name}_roped")
                nc.vector.tensor_add(rt[:], t1[:], t2[:])
                return rt

            for h in range(HPC):
                with nc.named_scope(f"head{h}"):
                    kT = load_rope(HPC + h, "k")
                    qT = load_rope(h, "q")
                    vraw = pb.tile([128, S], f32, tag="raw", bufs=4,
                                   name=f"vraw_{h}")
                    nc.sync.dma_start(
                        out=vraw[:],
                        in_=qkvT_d[:][(2 * HPC + h) * 128:
                                      (2 * HPC + h + 1) * 128, :])
                    v_sb = pb.tile([128, NKB, 128], f32r, tag="vsb", bufs=2,
                                   name=f"v_{h}")
                    for kb in range(NKB):
                        pst = psB.tile([128, 512], f32, tag="pss", bufs=3,
                                       name=f"ptr_{h}_{kb}")
                        nc.tensor.transpose(pst[0:128, 0:128],
                                            vraw[:, kb * 128:(kb + 1) * 128],
                                            ident[:])
                        nc.scalar.copy(v_sb[:, kb, :], pst[0:128, 0:128])

                    attn = pb.tile([128, S], f32r, tag="attn", bufs=2,
                                   name=f"attn_{h}")
                    for g in range(NG):
                        nkb = 4 * g + 4
                        po = psB.tile([128, 512], f32, tag="po", bufs=2,
                                      name=f"po_{h}_{g}")
                        pden = psB.tile([1, 512], f32, tag="pden", bufs=2,
                                        name=f"pden_{h}_{g}")
                        for kb in range(nkb):
                            pss = psB.tile([128, 512], f32, tag="pss", bufs=3,
                                           name=f"pss_{h}_{g}_{kb}")
                            nc.tensor.matmul(
                                pss[:], kT[:, kb * 128:(kb + 1) * 128],
                                qT[:, g * 512:(g + 1) * 512],
                                start=True, stop=True)
                            pt = pb.tile([128, 512], f32r, tag="pt", bufs=4,
                                         name=f"pt_{h}_{g}_{kb}")
                            nc.scalar.activation(pt[:], pss[:], AF.Exp,
                                                 scale=SCALE)
                            if kb >= 4 * g:
                                nc.vector.tensor_mul(pt[:], pt[:],
                                                     mask_sb[:, kb - 4 * g, :])
                            nc.tensor.matmul(pden[:], ones_r[:], pt[:],
                                             start=(kb == 0),
                                             stop=(kb == nkb - 1))
                            nc.tensor.matmul(po[:], v_sb[:, kb, :], pt[:],
                                             start=(kb == 0),
                                             stop=(kb == nkb - 1))
                        den1 = pb.tile([1, 512], f32, tag="den1", bufs=2,
                                       name=f"den1_{h}_{g}")
                        nc.scalar.copy(den1[:], pden[:])
                        rd1 = pb.tile([1, 512], f32, tag="rd1", bufs=2,
                                      name=f"rd1_{h}_{g}")
                        nc.vector.reciprocal(rd1[:], den1[:])
                        rden = pb.tile([128, 512], f32, tag="rden", bufs=2,
                                       name=f"rden_{h}_{g}")
                        nc.gpsimd.partition_broadcast(rden[:], rd1[:])
                        nc.vector.tensor_mul(attn[:, g * 512:(g + 1) * 512],
                                             po[:], rden[:])
                    nc.sync.dma_start(out=attn_d[:][h], in_=attn[:].bitcast(f32))
                    nc.gpsimd.collective_compute(
                        "AllGather", mybir.AluOpType.bypass,
                        replica_groups=GROUPS,
                        ins=[attn_d[:][h]], outs=[attn_ag[:][h]])

        # ---------------- stage C: o_proj (column-parallel) ----------------
        with nc.named_scope("stageC"), \
             tc.tile_pool(name="stC", bufs=1) as pc, \
             tc.tile_pool(name="psC", bufs=4, space="PSUM") as psC:
            wo_sb = pc.tile([128, NIB, JC], f32r, tag="wo", bufs=1)
            for d in range(8):
                nc.sync.dma_start(
                    out=wo_sb[:, 4 * d:4 * (d + 1), :],
                    in_=woT_v[:, 4 * d:4 * (d + 1), :].bitcast(f32r))
            for tb in range(NKB):
                at_sb = pc.tile([128, NIB, 128], f32r, tag="atC", bufs=2,
                                name=f"atC_{tb}")
                nc.sync.dma_start(
                    out=at_sb[:],
                    in_=ag_v[:, :, tb * 128:(tb + 1) * 128].bitcast(f32r))
                for mc in range(JC // 512):
                    psc = psC.tile([128, 512], f32, tag="psC",
                                   name=f"psC_{tb}_{mc}")
                    for jb in range(NIB):
                        nc.tensor.matmul(
                            psc[:], at_sb[:, jb, :],
                            wo_sb[:, jb, mc * 512:(mc + 1) * 512],
                            start=(jb == 0), stop=(jb == NIB - 1))
                    oc = pc.tile([128, 512], f32, tag="oC", bufs=4,
                                 name=f"oC_{tb}_{mc}")
                    nc.scalar.copy(oc[:], psc[:])
                    nc.sync.dma_start(
                        out=out[:][tb * 128:(tb + 1) * 128,
                                   mc * 512:(mc + 1) * 512],
                        in_=oc[:])

    nc.finalize()
    return nc


_NC_CACHE = None


def _get_nc():
    global _NC_CACHE
    if _NC_CACHE is None:
        _NC_CACHE = build_nc()
    return _NC_CACHE


def _host_inputs(hidden_states, positions, w_pack, w_o):
    hidden_states = np.asarray(hidden_states, dtype=np.float32)
    positions = np.asarray(positions)
    w_pack = np.asarray(w_pack, dtype=np.float32)
    w_o = np.asarray(w_o, dtype=np.float32)

    half = HD // 2
    inv_freq = (1.0 / (THETA ** (np.arange(half, dtype=np.float32) / half)))

    # causal mask variants for the 4 diagonal (128x512) tiles of a q-block
    masks = np.empty((4, 128, 512), dtype=np.float32)
    xs = np.arange(512)[None, :]
    ps = np.arange(128)[:, None]
    for v in range(4):
        masks[v] = (xs >= ps + 128 * v).astype(np.float32)

    in_maps = []
    for c in range(NCORES):
        b, r = divmod(c, TPN)
        heads = np.arange(HPC * r, HPC * (r + 1))
        rows = (heads[:, None] * HD + np.arange(HD)[None, :]).reshape(-1)
        w_core = np.concatenate(
            [w_pack[rows], w_pack[H + rows], w_pack[2 * H + rows]], axis=0)
        wT = np.ascontiguousarray(w_core.T)                      # [H, 3*JC]
        # o_proj m-shard rows, j-order permuted to match AllGather layout:
        # gathered row (h, r', d) holds global head 8*r'+h
        wo_shard = w_o[JC * r:JC * (r + 1), :]                   # [JC, H]
        woT_full = np.ascontiguousarray(wo_shard.T)              # [H=j, JC]
        woT_perm = woT_full.reshape(TPN, HPC, HD, JC) \
                           .transpose(1, 0, 2, 3).reshape(H, JC)
        hsT = np.ascontiguousarray(hidden_states[b].T)           # [H, S]
        ang = positions[b].astype(np.float32)[None, :] * inv_freq[:, None]
        cos_t = np.cos(ang).astype(np.float32)                   # [64, S]
        sin_t = np.sin(ang).astype(np.float32)
        cosf = np.concatenate([cos_t, cos_t], axis=0)            # [128, S]
        sinm = np.concatenate([-sin_t, sin_t], axis=0)
        in_maps.append({
            "hsT": hsT, "wT": wT, "woT": np.ascontiguousarray(woT_perm),
            "cosf": cosf, "sinm": sinm, "masks": masks,
        })
    return in_maps


def kernel(hidden_states, positions, w_pack, w_o):
    nc = _get_nc()
    in_maps = _host_inputs(hidden_states, positions, w_pack, w_o)
    res = run_bass_kernel_spmd(nc, in_maps, list(range(NCORES)))
    out = np.empty((B, S, H), dtype=np.float32)
    for c in range(NCORES):
        b, r = divmod(c, TPN)
        out[b][:, JC * r:JC * (r + 1)] = res.results[c]["out"]
    return out
